# revision 19
# baseline (speedup 1.0000x reference)
"""Deformable group-correlation kernel for TRN2 (8 NeuronCores).

Reference op: bilinear-sample right_feature at per-pixel coords
(base grid + flow + 1x9 window offsets + extra offsets), then group-wise
(4 groups x 64ch) mean of left * sampled -> (2, 36, 80, 160).

Sharding: 8 cores = (batch b in {0,1}) x (h-quarter hq in {0..3}).
Each core: all 256 channels, 20 pixel rows = 3200 pixels = 25 strips of 128.

Per-core pipeline (25 strips, 1152 samples each = 9 search-pos x 128 pixels):
  - host: zero-padded channel-last 4-corner patch table
    r4[(y,x), (c64,g,k)] bf16 (2KB granules) + precomputed gather indices
    (wrap, 16-partition wrapped) and bilinear weights (w4, bf16), so the
    device does no coordinate math.
  - gpsimd dma_gather (flat, alternating between 2 SWDGE queues): sample
    i = s*128+pp lands contiguously: patch[pp, s, 1024].
  - granule layout j = c*16 + g*4 + k -> channel-sum tree is contiguous
    halving adds.
  - DVE (critical path, ~10us/strip, all ops in bf16 2x mode): in-place
    patch *= lt, tree adds down to 16 (g,k) sums, * w4, sum over k ->
    corr[pp, (s,g)] f32.  Strips 1..24 run as fused PAIRS (double-FD
    instructions) to halve dispatch overhead.
  - scalar engine: replicates lt over the corner dim on-device (uploads
    shrink 6.5MB -> 1.6MB so the prologue gathers aren't starved) and
    DMAs corr out.
  - every semaphore has exactly ONE waiting engine, and the non-critical
    upload tail is paced behind the first lt replication batch.
"""

import sys

sys.path.insert(0, "/opt/trn_rl_repo")

from contextlib import ExitStack

import numpy as np
import ml_dtypes

from concourse import bacc, bass, mybir
from concourse.bass_utils import run_bass_kernel_spmd
from concourse.library_config import mlp as mlp_library

F32 = mybir.dt.float32
BF16 = mybir.dt.bfloat16
I16 = mybir.dt.int16
AF = mybir.AluOpType
AX = mybir.AxisListType
ACT = mybir.ActivationFunctionType

B, C, H, W = 2, 256, 80, 160
G, gC, S = 4, 64, 9
PADDING = 2
TAB_H, TAB_W = 84, 164  # table: y in [0,84), x in [0,164); row = y*164 + x
NROWS = TAB_H * TAB_W  # 13776 granule rows
ELEM = 4 * C  # 1024 bf16 per granule = 2KB (4 groups x 4 corners x 64 ch)
HQ = H // 4  # 20 rows per core
NSTRIP = HQ * W // 128  # 25 strips of 128 pixels
NI = S * 128  # 1152 samples per strip
NW = NI // 16  # 72 wrapped-index columns per strip
M = S * 16  # 144 (s, g, k) groups per strip, 64 channels each
MAGIC = 8388608.0  # 2**23

NSLOT = 7  # patch slots: 6 rotating (strips>=1) + 1 dedicated to strip 0
NPAIR = (NSTRIP - 1) // 2  # 12 fused strip-pairs (strips 1..24)

CH_A = 3  # wrap prologue chunk (strips 0..2)
LT_A = 6  # lt/w4 prologue chunk (strips 0..5)
_graph_cache = {}


def _build_graph():
    nc = bacc.Bacc(
        "TRN2",
        detect_race_conditions=False,
        num_swdge_queues=2,
        dynamic_dma_scratch_size=24576,
        target_bir_lowering=True,
    )

    r4 = nc.declare_dram_parameter("r4", [NROWS, ELEM], BF16, isOutput=False)
    ltc = nc.declare_dram_parameter("ltc", [128, NSTRIP * 256], BF16, isOutput=False)
    w4 = nc.declare_dram_parameter("w4", [128, NSTRIP * 36], BF16, isOutput=False)
    wrap = nc.declare_dram_parameter("wrap", [128, NSTRIP * NW], I16, isOutput=False)
    out = nc.declare_dram_parameter("out", [NSTRIP * 128, 36], F32, isOutput=True)

    with ExitStack() as stk:
        sb = lambda name, shape, dt: stk.enter_context(nc.sbuf_tensor(name, shape, dt))
        wrap_s = sb("wrap_s", [128, NSTRIP * NW], I16)
        ltc_s = sb("ltc_s", [128, NSTRIP * 256], BF16)
        lt_s = sb("lt_s", [128, NSTRIP * ELEM], BF16)
        w4_s = sb("w4_s", [128, NSTRIP * 36], BF16)
        patch = sb("patch", [128, NSLOT * S * ELEM], BF16)
        red = sb("red", [128, 2 * M], BF16)
        t3 = sb("t3", [128, 2 * M], BF16)
        corrS = [sb("corrS0", [128, 72], F32), sb("corrS1", [128, 72], F32)]
        scr = sb("scr", [128, 1], F32)
        sem = lambda name: stk.enter_context(nc.semaphore(name))
        # one waiting engine per semaphore:
        wrapA_sem = sem("wrapA_sem")  # gpsimd
        wrapB_sem = sem("wrapB_sem")  # gpsimd
        w4A_sem = sem("w4A_sem")  # vector
        w4B_sem = sem("w4B_sem")  # vector
        ltcA_sem = sem("ltcA_sem")  # scalar
        ltcB_sem = sem("ltcB_sem")  # scalar
        repl_sem = sem("repl_sem")  # vector (counts strips of lt replicated)
        pace_sem = sem("pace_sem")  # sync (gates upload tail)
        gat0A_sem = sem("gat0A_sem")  # vector
        gat0B_sem = sem("gat0B_sem")  # vector
        gat_sems = [sem(f"gat_sem{i}") for i in range(6)]  # vector
        dve_sem = sem("dve_sem")  # gpsimd (counts finished pairs)
        corr_sem = sem("corr_sem")  # scalar (counts finished output units)
        out_sems = [sem("out_sem0"), sem("out_sem1")]  # vector

        def pv(slot, off, dims):
            return bass.AP(patch, slot * S * ELEM + off, dims)

        PSTRIDE = NSLOT * S * ELEM  # patch partition stride (elements)

        with nc.Block() as block:

            @block.sync
            def _(sync):
                sync.dma_start(wrap_s[:, : CH_A * NW], wrap[:, : CH_A * NW]).then_inc(
                    wrapA_sem, 16
                )
                sync.dma_start(wrap_s[:, CH_A * NW :], wrap[:, CH_A * NW :]).then_inc(
                    wrapB_sem, 16
                )
                sync.dma_start(w4_s[:, : LT_A * 36], w4[:, : LT_A * 36]).then_inc(
                    w4A_sem, 16
                )
                sync.dma_start(
                    ltc_s[:, : LT_A * 256], ltc[:, : LT_A * 256]
                ).then_inc(ltcA_sem, 16)
                # tail paced behind the first replication batch so it never
                # competes with the strip-0..5 gathers
                sync.wait_ge(pace_sem, 1)
                sync.dma_start(
                    ltc_s[:, LT_A * 256 :], ltc[:, LT_A * 256 :]
                ).then_inc(ltcB_sem, 16)
                sync.dma_start(w4_s[:, LT_A * 36 :], w4[:, LT_A * 36 :]).then_inc(
                    w4B_sem, 16
                )

            @block.gpsimd
            def _(gpsimd):
                gpsimd.load_library(mlp_library)
                gpsimd.wait_ge(wrapA_sem, 16)

                # strip 0 (slot 6) in two halves so the DVE can start sooner
                dstA = pv(6, 0, [[PSTRIDE, 128], [ELEM, 4], [1, ELEM]])
                gpsimd.dma_gather(
                    dstA, r4[:, :], wrap_s[:, 0:32], 512, 512, ELEM,
                    transpose=False, single_packet=False, queue_num=0,
                ).then_inc(gat0A_sem, 16)
                dstB = pv(6, 4 * ELEM, [[PSTRIDE, 128], [ELEM, 5], [1, ELEM]])
                gpsimd.dma_gather(
                    dstB, r4[:, :], wrap_s[:, 32:72], 640, 640, ELEM,
                    transpose=False, single_packet=False, queue_num=1,
                ).then_inc(gat0B_sem, 16)

                for n in range(1, NSTRIP):
                    slot = (n - 1) % 6
                    if n == CH_A:
                        gpsimd.wait_ge(wrapB_sem, 16)
                    if n >= 7:
                        # slot previously held strip n-6 (pair (n-7)//2)
                        gpsimd.wait_ge(dve_sem, (n - 7) // 2 + 1)
                    dst = pv(slot, 0, [[PSTRIDE, 128], [ELEM, S], [1, ELEM]])
                    gpsimd.dma_gather(
                        dst,
                        r4[:, :],
                        wrap_s[:, n * NW : (n + 1) * NW],
                        NI,
                        NI,
                        ELEM,
                        transpose=False,
                        single_packet=False,
                        queue_num=n % 2,
                    ).then_inc(gat_sems[slot], 16)

            @block.vector
            def _(vector):
                vector.wait_ge(w4A_sem, 16)
                vector.wait_ge(repl_sem, 1)

                # ---- strip 0 (slot 6), two sub-blocks ----
                for sub in range(2):
                    s0, ns = (0, 4) if sub == 0 else (4, 5)
                    base = s0 * ELEM
                    vector.wait_ge(gat0A_sem if sub == 0 else gat0B_sem, 16)
                    o = pv(6, base, [[PSTRIDE, 128], [ELEM, ns], [1, ELEM]])
                    i1 = bass.AP(
                        lt_s, 0, [[NSTRIP * ELEM, 128], [0, ns], [1, ELEM]]
                    )
                    vector.tensor_tensor(out=o, in0=o, in1=i1, op=AF.mult)
                    for half in (512, 256, 128, 64, 32):
                        o = pv(6, base, [[PSTRIDE, 128], [ELEM, ns], [1, half]])
                        i1h = pv(
                            6, base + half, [[PSTRIDE, 128], [ELEM, ns], [1, half]]
                        )
                        vector.tensor_tensor(out=o, in0=o, in1=i1h, op=AF.add)
                    o = bass.AP(red, s0 * 16, [[2 * M, 128], [16, ns], [1, 16]])
                    i0 = pv(6, base, [[PSTRIDE, 128], [ELEM, ns], [1, 16]])
                    i1h = pv(6, base + 16, [[PSTRIDE, 128], [ELEM, ns], [1, 16]])
                    vector.tensor_tensor(out=o, in0=i0, in1=i1h, op=AF.add)
                    o = bass.AP(t3, s0 * 16, [[2 * M, 128], [16, ns], [4, 4], [1, 4]])
                    i0 = bass.AP(red, s0 * 16, [[2 * M, 128], [16, ns], [4, 4], [1, 4]])
                    i1h = bass.AP(
                        w4_s, s0 * 4, [[NSTRIP * 36, 128], [4, ns], [0, 4], [1, 4]]
                    )
                    vector.tensor_tensor(out=o, in0=i0, in1=i1h, op=AF.mult)
                    co = bass.AP(corrS[0], s0 * 4, [[72, 128], [1, ns * 4]])
                    ti = bass.AP(t3, s0 * 16, [[2 * M, 128], [4, ns * 4], [1, 4]])
                    mm = vector.tensor_reduce(co, ti, axis=AX.X, op=AF.add)
                    if sub == 1:
                        mm.then_inc(corr_sem, 1)

                # ---- strips 1..24 as fused pairs ----
                for p in range(NPAIR):
                    a = 2 * p + 1  # first strip of the pair
                    sl = (2 * p) % 6  # slot of strip a; strip a+1 is sl+1
                    vector.wait_ge(repl_sem, 2 * p + 3)
                    if a == 5:
                        vector.wait_ge(w4B_sem, 16)
                    u = (p // 3) + 1  # uses of this slot pair so far
                    vector.wait_ge(gat_sems[sl], 16 * u)
                    vector.wait_ge(gat_sems[sl + 1], 16 * u)
                    # patch *= lt (in place), both strips in one instruction
                    o = pv(
                        sl, 0, [[PSTRIDE, 128], [S * ELEM, 2], [ELEM, S], [1, ELEM]]
                    )
                    i1 = bass.AP(
                        lt_s,
                        a * ELEM,
                        [[NSTRIP * ELEM, 128], [ELEM, 2], [0, S], [1, ELEM]],
                    )
                    vector.tensor_tensor(out=o, in0=o, in1=i1, op=AF.mult)
                    # tree: contiguous halves 512 ... 32
                    for half in (512, 256, 128, 64, 32):
                        o = pv(sl, 0, [[PSTRIDE, 128], [ELEM, 2 * S], [1, half]])
                        i1 = pv(sl, half, [[PSTRIDE, 128], [ELEM, 2 * S], [1, half]])
                        vector.tensor_tensor(out=o, in0=o, in1=i1, op=AF.add)
                    # final pair -> bf16 red[pp, (strip, s, gk)]; slots free
                    o = bass.AP(red, 0, [[2 * M, 128], [16, 2 * S], [1, 16]])
                    i0 = pv(sl, 0, [[PSTRIDE, 128], [ELEM, 2 * S], [1, 16]])
                    i1 = pv(sl, 16, [[PSTRIDE, 128], [ELEM, 2 * S], [1, 16]])
                    vector.tensor_tensor(out=o, in0=i0, in1=i1, op=AF.add).then_inc(
                        dve_sem, 1
                    )
                    # t3 = red * w4 (w4 broadcast over g), bf16 2x
                    o = bass.AP(t3, 0, [[2 * M, 128], [16, 2 * S], [4, 4], [1, 4]])
                    i0 = bass.AP(red, 0, [[2 * M, 128], [16, 2 * S], [4, 4], [1, 4]])
                    i1 = bass.AP(
                        w4_s, a * 36, [[NSTRIP * 36, 128], [4, 2 * S], [0, 4], [1, 4]]
                    )
                    vector.tensor_tensor(out=o, in0=i0, in1=i1, op=AF.mult)
                    # corr[pp, (strip, s, g)] = sum_k t3
                    un = p + 1  # output unit index (0 = strip 0)
                    if un >= 2:
                        vector.wait_ge(out_sems[un % 2], 16 * ((un - 2) // 2 + 1))
                    co = bass.AP(corrS[un % 2], 0, [[72, 128], [1, 72]])
                    ti = bass.AP(t3, 0, [[2 * M, 128], [16, 2 * S], [4, 4], [1, 4]])
                    vector.tensor_reduce(co, ti, axis=AX.X, op=AF.add).then_inc(
                        corr_sem, 1
                    )

            @block.scalar
            def _(scalar):
                # replicate lt over the corner dim: lt_s[pp, n, c, g, k] =
                # ltc_s[pp, n, c, g] (k broadcast), one activation per strip
                def repl(n):
                    o = bass.AP(
                        lt_s, n * ELEM, [[NSTRIP * ELEM, 128], [16, 64], [4, 4], [1, 4]]
                    )
                    i = bass.AP(
                        ltc_s, n * 256, [[NSTRIP * 256, 128], [4, 64], [1, 4], [0, 4]]
                    )
                    return scalar.activation(o, i, ACT.Copy)

                scalar.wait_ge(ltcA_sem, 16)
                for n in range(LT_A):
                    repl(n).then_inc(repl_sem, 1)
                scalar.activation(scr[:, :], scr[:, :], ACT.Copy).then_inc(pace_sem, 1)
                scalar.wait_ge(ltcB_sem, 16)
                for n in range(LT_A, NSTRIP):
                    repl(n).then_inc(repl_sem, 1)

                # output units: strip 0, then the 12 pairs
                scalar.wait_ge(corr_sem, 1)
                scalar.dma_start(out[0:128, :], corrS[0][:, 0:36]).then_inc(
                    out_sems[0], 16
                )
                for un in range(1, NPAIR + 1):
                    p = un - 1
                    scalar.wait_ge(corr_sem, un + 1)
                    dst = bass.AP(
                        out,
                        (2 * p + 1) * 128 * 36,
                        [[36, 128], [128 * 36, 2], [1, 36]],
                    )
                    src = bass.AP(corrS[un % 2], 0, [[72, 128], [36, 2], [1, 36]])
                    scalar.dma_start(dst, src).then_inc(out_sems[un % 2], 16)
                scalar.wait_ge(out_sems[0], 16 * ((NPAIR + 2) // 2))
                scalar.wait_ge(out_sems[1], 16 * ((NPAIR + 1) // 2))

    if not nc.is_finalized():
        nc.finalize()
    return nc


def _host_prep(left_feature, right_feature, flow, extra_offset):
    """Per-core inputs. Core ordering: core = b*4 + hq."""
    lf = np.asarray(left_feature, np.float32)
    rf = np.asarray(right_feature, np.float32)
    fl = np.asarray(flow, np.float32)
    eo = np.asarray(extra_offset, np.float32)

    p_idx = np.arange(128)
    strip = np.arange(NSTRIP)
    pi = strip[:, None] * 128 + p_idx[None, :]  # [25, 128] pixel within quarter
    hl = pi // W
    w = pi % W

    offx = np.arange(S, dtype=np.float32) - 4.0

    in_maps = []
    for b in range(B):
        rp = np.zeros((TAB_H + 1, TAB_W + 1, C), np.float32)
        rp[PADDING : PADDING + H, PADDING : PADDING + W] = rf[b].transpose(1, 2, 0)
        # corners k: (dy,dx) = (k//2, k%2); row layout (c64, g, k)
        corn = np.stack(
            [
                rp[0:TAB_H, 0:TAB_W],
                rp[0:TAB_H, 1 : TAB_W + 1],
                rp[1 : TAB_H + 1, 0:TAB_W],
                rp[1 : TAB_H + 1, 1 : TAB_W + 1],
            ],
            axis=2,
        )  # [84, 164, 4k, 256c]
        r4_np = np.ascontiguousarray(
            corn.reshape(TAB_H, TAB_W, 4, G, gC)
            .transpose(0, 1, 4, 3, 2)  # (y, x, c, g, k)
            .reshape(NROWS, ELEM)
            .astype(ml_dtypes.bfloat16)
        )

        for hq in range(4):
            h = hq * HQ + hl  # [25, 128] global h
            fx = fl[b, 0][h, w]
            fy = fl[b, 1][h, w]
            cbx = w.astype(np.float32) + fx + PADDING  # [25, 128]
            cby = h.astype(np.float32) + fy + PADDING

            eo_b = eo[b].reshape(S, 2, H, W)
            exx = eo_b[:, 0][:, h, w] + offx[:, None, None]  # [S, 25, 128]
            exy = eo_b[:, 1][:, h, w]

            hflat = h.reshape(-1)
            wflat = w.reshape(-1)
            # ltc[pp, strip, (c, g)] = left[b, g*64+c, pix]/64
            lv = (lf[b] / gC)[:, hflat, wflat]  # [256, 3200]
            ltc_np = np.ascontiguousarray(
                lv.reshape(G, gC, NSTRIP, 128)
                .transpose(3, 2, 1, 0)  # [128, 25, c, g]
                .reshape(128, NSTRIP * 256)
                .astype(ml_dtypes.bfloat16)
            )

            # coords in f32 (round-to-nearest via the 2^23 trick)
            # [128p, strip, s]
            xq = np.clip(
                exx.transpose(2, 1, 0) + cbx.T[:, :, None], 0.5, TAB_W - 1.5
            ).astype(np.float32)
            yq = np.clip(
                exy.transpose(2, 1, 0) + cby.T[:, :, None], 0.5, TAB_H - 1.5
            ).astype(np.float32)
            x0 = ((xq + np.float32(MAGIC - 0.5)) + np.float32(-MAGIC)).astype(
                np.float32
            )
            y0 = ((yq + np.float32(MAGIC - 0.5)) + np.float32(-MAGIC)).astype(
                np.float32
            )
            fxw, fyw = xq - x0, yq - y0  # [128p, strip, s]
            gxw, gyw = 1.0 - fxw, 1.0 - fyw
            w4v = np.stack(
                [gxw * gyw, fxw * gyw, gxw * fyw, fxw * fyw], 0
            )  # [4k, 128, strip, s]
            # w4[pp, strip, s, k]
            w4_np = np.ascontiguousarray(
                w4v.transpose(1, 2, 3, 0)
                .reshape(128, NSTRIP * 36)
                .astype(ml_dtypes.bfloat16)
            )

            # gather row index = y0*TAB_W + x0, wrapped: idx for i=s*128+pp at
            # [pp%16, strip*NW + s*8 + pp//16], replicated over 8 Q7 cores.
            idx = (y0 * np.float32(TAB_W) + x0).astype(np.int32)  # [128,strip,s]
            idx_r = idx.reshape(8, 16, NSTRIP, S)  # [a=pp//16, m=pp%16, n, s]
            wrap_np = np.ascontiguousarray(
                np.tile(
                    idx_r.transpose(1, 2, 3, 0).reshape(16, NSTRIP * NW), (8, 1)
                ).astype(np.int16)
            )

            in_maps.append(
                {
                    "r4": r4_np,
                    "ltc": ltc_np,
                    "w4": w4_np,
                    "wrap": wrap_np,
                }
            )
    return in_maps


def kernel(**inputs):
    if "nc" not in _graph_cache:
        _graph_cache["nc"] = _build_graph()
    nc = _graph_cache["nc"]

    in_maps = _host_prep(
        inputs["left_feature"],
        inputs["right_feature"],
        inputs["flow"],
        inputs["extra_offset"],
    )
    res = run_bass_kernel_spmd(nc, in_maps, core_ids=list(range(8)))
    _graph_cache["last_res"] = res
    outs = [r["out"] for r in res.results]

    full = np.zeros((B, G * S, H, W), np.float32)
    for core in range(8):
        b, hq = divmod(core, 4)
        # out rows: [strip, pp], cols: [s, g]
        o = np.asarray(outs[core], np.float32).reshape(NSTRIP, 128, S, G)
        o = o.transpose(3, 2, 0, 1).reshape(G, S, HQ, W)
        for g in range(G):
            for s in range(S):
                full[b, g * S + s, hq * HQ : (hq + 1) * HQ, :] = o[g, s]
    return full


# revision 20
# speedup vs baseline: 1.1350x; 1.1350x over previous
"""Deformable group-correlation kernel for TRN2 (8 NeuronCores).

Reference op: bilinear-sample right_feature at per-pixel coords
(base grid + flow + 1x9 window offsets + extra offsets), then group-wise
(4 groups x 64ch) mean of left * sampled -> (2, 36, 80, 160).

Sharding: 8 cores = (batch b in {0,1}) x (h-quarter hq in {0..3}).
Each core: all 256 channels, 20 pixel rows = 3200 pixels = 25 strips of 128.

Per-core pipeline (25 strips, 1152 samples each = 9 search-pos x 128 pixels):
  - host: zero-padded channel-last 4-corner patch table
    r4[(y,x), (c64,g,k)] bf16 (2KB granules) + precomputed gather indices
    (wrap, 16-partition wrapped) and bilinear weights (w4, bf16), so the
    device does no coordinate math.
  - gpsimd dma_gather (flat, alternating between 2 SWDGE queues): sample
    i = s*128+pp lands contiguously: patch[pp, s, 1024].
  - granule layout j = c*16 + g*4 + k -> channel-sum tree is contiguous
    halving adds.
  - DVE (critical path, ~10us/strip, all ops in bf16 2x mode): in-place
    patch *= lt, tree adds down to 16 (g,k) sums, * w4, sum over k ->
    corr[pp, (s,g)] f32.  Strips 1..24 run as fused PAIRS (double-FD
    instructions) to halve dispatch overhead.
  - scalar engine: replicates lt over the corner dim on-device (uploads
    shrink 6.5MB -> 1.6MB so the prologue gathers aren't starved) and
    DMAs corr out.
  - every semaphore has exactly ONE waiting engine, and the non-critical
    upload tail is paced behind the first lt replication batch.
"""

import sys

sys.path.insert(0, "/opt/trn_rl_repo")

from contextlib import ExitStack

import numpy as np
import ml_dtypes

from concourse import bacc, bass, mybir
from concourse.bass_utils import run_bass_kernel_spmd
from concourse.library_config import mlp as mlp_library

F32 = mybir.dt.float32
BF16 = mybir.dt.bfloat16
I16 = mybir.dt.int16
AF = mybir.AluOpType
AX = mybir.AxisListType
ACT = mybir.ActivationFunctionType

B, C, H, W = 2, 256, 80, 160
G, gC, S = 4, 64, 9
PADDING = 2
TAB_H, TAB_W = 84, 164  # table: y in [0,84), x in [0,164); row = y*164 + x
NROWS = TAB_H * TAB_W  # 13776 granule rows
ELEM = 4 * C  # 1024 bf16 per granule = 2KB (4 groups x 4 corners x 64 ch)
HQ = H // 4  # 20 rows per core
NSTRIP = HQ * W // 128  # 25 strips of 128 pixels
NI = S * 128  # 1152 samples per strip
NW = NI // 16  # 72 wrapped-index columns per strip
M = S * 16  # 144 (s, g, k) groups per strip, 64 channels each
MAGIC = 8388608.0  # 2**23

NSLOT = 7  # patch slots: 6 rotating (strips>=1) + 1 dedicated to strip 0
NPAIR = (NSTRIP - 1) // 2  # 12 fused strip-pairs (strips 1..24)

CH_A = 3  # wrap prologue chunk (strips 0..2)
LT_A = 6  # lt/w4 prologue chunk (strips 0..5)
_graph_cache = {}


def _build_graph():
    nc = bacc.Bacc(
        "TRN2",
        detect_race_conditions=False,
        num_swdge_queues=2,
        target_bir_lowering=True,
    )

    r4 = nc.declare_dram_parameter("r4", [NROWS, ELEM], BF16, isOutput=False)
    ltc = nc.declare_dram_parameter("ltc", [128, NSTRIP * 256], BF16, isOutput=False)
    w4 = nc.declare_dram_parameter("w4", [128, NSTRIP * 36], BF16, isOutput=False)
    wrap = nc.declare_dram_parameter("wrap", [128, NSTRIP * NW], I16, isOutput=False)
    out = nc.declare_dram_parameter("out", [NSTRIP * 128, 36], F32, isOutput=True)

    with ExitStack() as stk:
        sb = lambda name, shape, dt: stk.enter_context(nc.sbuf_tensor(name, shape, dt))
        wrap_s = sb("wrap_s", [128, NSTRIP * NW], I16)
        ltc_s = sb("ltc_s", [128, NSTRIP * 256], BF16)
        lt_s = sb("lt_s", [128, NSTRIP * ELEM], BF16)
        w4_s = sb("w4_s", [128, NSTRIP * 36], BF16)
        patch = sb("patch", [128, NSLOT * S * ELEM], BF16)
        red = sb("red", [128, 2 * M], BF16)
        t3 = sb("t3", [128, 2 * M], BF16)
        corrS = [sb("corrS0", [128, 72], F32), sb("corrS1", [128, 72], F32)]
        scr = sb("scr", [128, 1], F32)
        sem = lambda name: stk.enter_context(nc.semaphore(name))
        # one waiting engine per semaphore:
        wrapA_sem = sem("wrapA_sem")  # gpsimd
        wrapB_sem = sem("wrapB_sem")  # gpsimd
        w4A_sem = sem("w4A_sem")  # vector
        w4B_sem = sem("w4B_sem")  # vector
        ltcA_sem = sem("ltcA_sem")  # scalar
        ltcB_sem = sem("ltcB_sem")  # scalar
        repl_sem = sem("repl_sem")  # vector (counts strips of lt replicated)
        pace_sem = sem("pace_sem")  # sync (gates upload tail)
        gat0A_sem = sem("gat0A_sem")  # vector
        gat0B_sem = sem("gat0B_sem")  # vector
        gat_sems = [sem(f"gat_sem{i}") for i in range(6)]  # vector
        dve_sem = sem("dve_sem")  # gpsimd (counts finished pairs)
        corr_sem = sem("corr_sem")  # scalar (counts finished output units)
        out_sems = [sem("out_sem0"), sem("out_sem1")]  # vector

        def pv(slot, off, dims):
            return bass.AP(patch, slot * S * ELEM + off, dims)

        PSTRIDE = NSLOT * S * ELEM  # patch partition stride (elements)

        with nc.Block() as block:

            @block.sync
            def _(sync):
                sync.dma_start(wrap_s[:, : CH_A * NW], wrap[:, : CH_A * NW]).then_inc(
                    wrapA_sem, 16
                )
                sync.dma_start(wrap_s[:, CH_A * NW :], wrap[:, CH_A * NW :]).then_inc(
                    wrapB_sem, 16
                )
                sync.dma_start(w4_s[:, : LT_A * 36], w4[:, : LT_A * 36]).then_inc(
                    w4A_sem, 16
                )
                sync.dma_start(
                    ltc_s[:, : LT_A * 256], ltc[:, : LT_A * 256]
                ).then_inc(ltcA_sem, 16)
                # tail paced behind the first replication batch so it never
                # competes with the strip-0..5 gathers
                sync.wait_ge(pace_sem, 1)
                sync.dma_start(
                    ltc_s[:, LT_A * 256 :], ltc[:, LT_A * 256 :]
                ).then_inc(ltcB_sem, 16)
                sync.dma_start(w4_s[:, LT_A * 36 :], w4[:, LT_A * 36 :]).then_inc(
                    w4B_sem, 16
                )

            @block.gpsimd
            def _(gpsimd):
                gpsimd.load_library(mlp_library)
                gpsimd.wait_ge(wrapA_sem, 16)

                # strip 0 (slot 6) in two halves so the DVE can start sooner
                dstA = pv(6, 0, [[PSTRIDE, 128], [ELEM, 4], [1, ELEM]])
                gpsimd.dma_gather(
                    dstA, r4[:, :], wrap_s[:, 0:32], 512, 512, ELEM,
                    transpose=False, single_packet=False, queue_num=0,
                ).then_inc(gat0A_sem, 16)
                dstB = pv(6, 4 * ELEM, [[PSTRIDE, 128], [ELEM, 5], [1, ELEM]])
                gpsimd.dma_gather(
                    dstB, r4[:, :], wrap_s[:, 32:72], 640, 640, ELEM,
                    transpose=False, single_packet=False, queue_num=1,
                ).then_inc(gat0B_sem, 16)

                for n in range(1, NSTRIP):
                    slot = (n - 1) % 6
                    if n == CH_A:
                        gpsimd.wait_ge(wrapB_sem, 16)
                    if n >= 7:
                        # slot previously held strip n-6 (pair (n-7)//2)
                        gpsimd.wait_ge(dve_sem, (n - 7) // 2 + 1)
                    dst = pv(slot, 0, [[PSTRIDE, 128], [ELEM, S], [1, ELEM]])
                    gpsimd.dma_gather(
                        dst,
                        r4[:, :],
                        wrap_s[:, n * NW : (n + 1) * NW],
                        NI,
                        NI,
                        ELEM,
                        transpose=False,
                        single_packet=False,
                        queue_num=n % 2,
                    ).then_inc(gat_sems[slot], 16)

            @block.vector
            def _(vector):
                vector.wait_ge(w4A_sem, 16)
                vector.wait_ge(repl_sem, 1)

                # ---- strip 0 (slot 6), two sub-blocks ----
                for sub in range(2):
                    s0, ns = (0, 4) if sub == 0 else (4, 5)
                    base = s0 * ELEM
                    vector.wait_ge(gat0A_sem if sub == 0 else gat0B_sem, 16)
                    o = pv(6, base, [[PSTRIDE, 128], [ELEM, ns], [1, ELEM]])
                    i1 = bass.AP(
                        lt_s, 0, [[NSTRIP * ELEM, 128], [0, ns], [1, ELEM]]
                    )
                    vector.tensor_tensor(out=o, in0=o, in1=i1, op=AF.mult)
                    for half in (512, 256, 128, 64, 32):
                        o = pv(6, base, [[PSTRIDE, 128], [ELEM, ns], [1, half]])
                        i1h = pv(
                            6, base + half, [[PSTRIDE, 128], [ELEM, ns], [1, half]]
                        )
                        vector.tensor_tensor(out=o, in0=o, in1=i1h, op=AF.add)
                    o = bass.AP(red, s0 * 16, [[2 * M, 128], [16, ns], [1, 16]])
                    i0 = pv(6, base, [[PSTRIDE, 128], [ELEM, ns], [1, 16]])
                    i1h = pv(6, base + 16, [[PSTRIDE, 128], [ELEM, ns], [1, 16]])
                    vector.tensor_tensor(out=o, in0=i0, in1=i1h, op=AF.add)
                    o = bass.AP(t3, s0 * 16, [[2 * M, 128], [16, ns], [4, 4], [1, 4]])
                    i0 = bass.AP(red, s0 * 16, [[2 * M, 128], [16, ns], [4, 4], [1, 4]])
                    i1h = bass.AP(
                        w4_s, s0 * 4, [[NSTRIP * 36, 128], [4, ns], [0, 4], [1, 4]]
                    )
                    vector.tensor_tensor(out=o, in0=i0, in1=i1h, op=AF.mult)
                    co = bass.AP(corrS[0], s0 * 4, [[72, 128], [1, ns * 4]])
                    ti = bass.AP(t3, s0 * 16, [[2 * M, 128], [4, ns * 4], [1, 4]])
                    mm = vector.tensor_reduce(co, ti, axis=AX.X, op=AF.add)
                    if sub == 1:
                        mm.then_inc(corr_sem, 1)

                # ---- strips 1..24 as fused pairs ----
                for p in range(NPAIR):
                    a = 2 * p + 1  # first strip of the pair
                    sl = (2 * p) % 6  # slot of strip a; strip a+1 is sl+1
                    vector.wait_ge(repl_sem, 2 * p + 3)
                    if a == 5:
                        vector.wait_ge(w4B_sem, 16)
                    u = (p // 3) + 1  # uses of this slot pair so far
                    vector.wait_ge(gat_sems[sl], 16 * u)
                    vector.wait_ge(gat_sems[sl + 1], 16 * u)
                    # patch *= lt (in place), both strips in one instruction
                    o = pv(
                        sl, 0, [[PSTRIDE, 128], [S * ELEM, 2], [ELEM, S], [1, ELEM]]
                    )
                    i1 = bass.AP(
                        lt_s,
                        a * ELEM,
                        [[NSTRIP * ELEM, 128], [ELEM, 2], [0, S], [1, ELEM]],
                    )
                    vector.tensor_tensor(out=o, in0=o, in1=i1, op=AF.mult)
                    # tree: contiguous halves 512 ... 32
                    for half in (512, 256, 128, 64, 32):
                        o = pv(sl, 0, [[PSTRIDE, 128], [ELEM, 2 * S], [1, half]])
                        i1 = pv(sl, half, [[PSTRIDE, 128], [ELEM, 2 * S], [1, half]])
                        vector.tensor_tensor(out=o, in0=o, in1=i1, op=AF.add)
                    # final pair -> bf16 red[pp, (strip, s, gk)]; slots free
                    o = bass.AP(red, 0, [[2 * M, 128], [16, 2 * S], [1, 16]])
                    i0 = pv(sl, 0, [[PSTRIDE, 128], [ELEM, 2 * S], [1, 16]])
                    i1 = pv(sl, 16, [[PSTRIDE, 128], [ELEM, 2 * S], [1, 16]])
                    vector.tensor_tensor(out=o, in0=i0, in1=i1, op=AF.add).then_inc(
                        dve_sem, 1
                    )
                    # t3 = red * w4 (w4 broadcast over g), bf16 2x
                    o = bass.AP(t3, 0, [[2 * M, 128], [16, 2 * S], [4, 4], [1, 4]])
                    i0 = bass.AP(red, 0, [[2 * M, 128], [16, 2 * S], [4, 4], [1, 4]])
                    i1 = bass.AP(
                        w4_s, a * 36, [[NSTRIP * 36, 128], [4, 2 * S], [0, 4], [1, 4]]
                    )
                    vector.tensor_tensor(out=o, in0=i0, in1=i1, op=AF.mult)
                    # corr[pp, (strip, s, g)] = sum_k t3
                    un = p + 1  # output unit index (0 = strip 0)
                    if un >= 2:
                        vector.wait_ge(out_sems[un % 2], 16 * ((un - 2) // 2 + 1))
                    co = bass.AP(corrS[un % 2], 0, [[72, 128], [1, 72]])
                    ti = bass.AP(t3, 0, [[2 * M, 128], [16, 2 * S], [4, 4], [1, 4]])
                    vector.tensor_reduce(co, ti, axis=AX.X, op=AF.add).then_inc(
                        corr_sem, 1
                    )

            @block.scalar
            def _(scalar):
                # replicate lt over the corner dim: lt_s[pp, n, c, g, k] =
                # ltc_s[pp, n, c, g] (k broadcast), one activation per strip
                def repl(n):
                    o = bass.AP(
                        lt_s, n * ELEM, [[NSTRIP * ELEM, 128], [16, 64], [4, 4], [1, 4]]
                    )
                    i = bass.AP(
                        ltc_s, n * 256, [[NSTRIP * 256, 128], [4, 64], [1, 4], [0, 4]]
                    )
                    return scalar.activation(o, i, ACT.Copy)

                scalar.wait_ge(ltcA_sem, 16)
                for n in range(LT_A):
                    repl(n).then_inc(repl_sem, 1)
                scalar.activation(scr[:, :], scr[:, :], ACT.Copy).then_inc(pace_sem, 1)
                scalar.wait_ge(ltcB_sem, 16)
                for n in range(LT_A, NSTRIP):
                    repl(n).then_inc(repl_sem, 1)

                # output units: strip 0, then the 12 pairs
                scalar.wait_ge(corr_sem, 1)
                scalar.dma_start(out[0:128, :], corrS[0][:, 0:36]).then_inc(
                    out_sems[0], 16
                )
                for un in range(1, NPAIR + 1):
                    p = un - 1
                    scalar.wait_ge(corr_sem, un + 1)
                    dst = bass.AP(
                        out,
                        (2 * p + 1) * 128 * 36,
                        [[36, 128], [128 * 36, 2], [1, 36]],
                    )
                    src = bass.AP(corrS[un % 2], 0, [[72, 128], [36, 2], [1, 36]])
                    scalar.dma_start(dst, src).then_inc(out_sems[un % 2], 16)
                scalar.wait_ge(out_sems[0], 16 * ((NPAIR + 2) // 2))
                scalar.wait_ge(out_sems[1], 16 * ((NPAIR + 1) // 2))

    if not nc.is_finalized():
        nc.finalize()
    return nc


def _host_prep(left_feature, right_feature, flow, extra_offset):
    """Per-core inputs. Core ordering: core = b*4 + hq."""
    lf = np.asarray(left_feature, np.float32)
    rf = np.asarray(right_feature, np.float32)
    fl = np.asarray(flow, np.float32)
    eo = np.asarray(extra_offset, np.float32)

    p_idx = np.arange(128)
    strip = np.arange(NSTRIP)
    pi = strip[:, None] * 128 + p_idx[None, :]  # [25, 128] pixel within quarter
    hl = pi // W
    w = pi % W

    offx = np.arange(S, dtype=np.float32) - 4.0

    in_maps = []
    for b in range(B):
        rp = np.zeros((TAB_H + 1, TAB_W + 1, C), np.float32)
        rp[PADDING : PADDING + H, PADDING : PADDING + W] = rf[b].transpose(1, 2, 0)
        # corners k: (dy,dx) = (k//2, k%2); row layout (c64, g, k)
        corn = np.stack(
            [
                rp[0:TAB_H, 0:TAB_W],
                rp[0:TAB_H, 1 : TAB_W + 1],
                rp[1 : TAB_H + 1, 0:TAB_W],
                rp[1 : TAB_H + 1, 1 : TAB_W + 1],
            ],
            axis=2,
        )  # [84, 164, 4k, 256c]
        r4_np = np.ascontiguousarray(
            corn.reshape(TAB_H, TAB_W, 4, G, gC)
            .transpose(0, 1, 4, 3, 2)  # (y, x, c, g, k)
            .reshape(NROWS, ELEM)
            .astype(ml_dtypes.bfloat16)
        )

        for hq in range(4):
            h = hq * HQ + hl  # [25, 128] global h
            fx = fl[b, 0][h, w]
            fy = fl[b, 1][h, w]
            cbx = w.astype(np.float32) + fx + PADDING  # [25, 128]
            cby = h.astype(np.float32) + fy + PADDING

            eo_b = eo[b].reshape(S, 2, H, W)
            exx = eo_b[:, 0][:, h, w] + offx[:, None, None]  # [S, 25, 128]
            exy = eo_b[:, 1][:, h, w]

            hflat = h.reshape(-1)
            wflat = w.reshape(-1)
            # ltc[pp, strip, (c, g)] = left[b, g*64+c, pix]/64
            lv = (lf[b] / gC)[:, hflat, wflat]  # [256, 3200]
            ltc_np = np.ascontiguousarray(
                lv.reshape(G, gC, NSTRIP, 128)
                .transpose(3, 2, 1, 0)  # [128, 25, c, g]
                .reshape(128, NSTRIP * 256)
                .astype(ml_dtypes.bfloat16)
            )

            # coords in f32 (round-to-nearest via the 2^23 trick)
            # [128p, strip, s]
            xq = np.clip(
                exx.transpose(2, 1, 0) + cbx.T[:, :, None], 0.5, TAB_W - 1.5
            ).astype(np.float32)
            yq = np.clip(
                exy.transpose(2, 1, 0) + cby.T[:, :, None], 0.5, TAB_H - 1.5
            ).astype(np.float32)
            x0 = ((xq + np.float32(MAGIC - 0.5)) + np.float32(-MAGIC)).astype(
                np.float32
            )
            y0 = ((yq + np.float32(MAGIC - 0.5)) + np.float32(-MAGIC)).astype(
                np.float32
            )
            fxw, fyw = xq - x0, yq - y0  # [128p, strip, s]
            gxw, gyw = 1.0 - fxw, 1.0 - fyw
            w4v = np.stack(
                [gxw * gyw, fxw * gyw, gxw * fyw, fxw * fyw], 0
            )  # [4k, 128, strip, s]
            # w4[pp, strip, s, k]
            w4_np = np.ascontiguousarray(
                w4v.transpose(1, 2, 3, 0)
                .reshape(128, NSTRIP * 36)
                .astype(ml_dtypes.bfloat16)
            )

            # gather row index = y0*TAB_W + x0, wrapped: idx for i=s*128+pp at
            # [pp%16, strip*NW + s*8 + pp//16], replicated over 8 Q7 cores.
            idx = (y0 * np.float32(TAB_W) + x0).astype(np.int32)  # [128,strip,s]
            idx_r = idx.reshape(8, 16, NSTRIP, S)  # [a=pp//16, m=pp%16, n, s]
            wrap_np = np.ascontiguousarray(
                np.tile(
                    idx_r.transpose(1, 2, 3, 0).reshape(16, NSTRIP * NW), (8, 1)
                ).astype(np.int16)
            )

            in_maps.append(
                {
                    "r4": r4_np,
                    "ltc": ltc_np,
                    "w4": w4_np,
                    "wrap": wrap_np,
                }
            )
    return in_maps


def kernel(**inputs):
    if "nc" not in _graph_cache:
        _graph_cache["nc"] = _build_graph()
    nc = _graph_cache["nc"]

    in_maps = _host_prep(
        inputs["left_feature"],
        inputs["right_feature"],
        inputs["flow"],
        inputs["extra_offset"],
    )
    res = run_bass_kernel_spmd(nc, in_maps, core_ids=list(range(8)))
    _graph_cache["last_res"] = res
    outs = [r["out"] for r in res.results]

    full = np.zeros((B, G * S, H, W), np.float32)
    for core in range(8):
        b, hq = divmod(core, 4)
        # out rows: [strip, pp], cols: [s, g]
        o = np.asarray(outs[core], np.float32).reshape(NSTRIP, 128, S, G)
        o = o.transpose(3, 2, 0, 1).reshape(G, S, HQ, W)
        for g in range(G):
            for s in range(S):
                full[b, g * S + s, hq * HQ : (hq + 1) * HQ, :] = o[g, s]
    return full
